# revision 63
# baseline (speedup 1.0000x reference)
# Trainium2 Bass kernel for the Chebyshev-GCN GRU decoder (gnn_message_passing).
#
# Problem: B=16, N=2048, F=64, K=2 Chebyshev taps, T=8 decode steps.
#   per step: gates = cheb(L, [x, hx]) @ W_gate; r,u = sigmoid(gates)
#             cy = tanh(cheb(L, [x, r*hx]) @ W_upd); hy = u*hx + (1-u)*cy
#             yt = sigmoid(hy @ W_edge)
#
# Strategy (373us -> 168.2us vs the fp32r baseline; HW rel err 1.32e-2):
#  - Data-parallel over batch: 8 cores x 2 batches each; c = b*64+f = 128
#    partitions for all "transposed"-layout [c, n] tensors.
#  - Big matmuls (L@hx, L@(r*hx)) run as fp8e4m3 DoubleRow: stationary =
#    fp8 hx_nat m-tile pairs [128, 2, 128], moving = fp8 L^T [128, 2, 256].
#    One instruction contracts K=256 at 0.5 cycles/row -> 4x the fp32r
#    rate.  L is pre-scaled x64 on host, hx x16 on device; all scales are
#    folded into the fp8 gate weights.  Each matmul is emitted as two
#    half-psum passes (all K block-groups x chunks 0-3, then x chunks 4-7)
#    so the half-0 cast and its consumers overlap the half-1 pass, while
#    the input side pipelines on the per-block stationary casts.
#  - Gate/cand hx+lxh matmul pairs are also fp8 DoubleRow: [wh0|wh1]
#    stationary pairs against [hxT8|lxh8] packed adjacently in one SBUF
#    tile so a single 3D AP covers both slots.  Step-invariant gate
#    constants stay EXACT (bf16, G-prescaled, bias folded in): added into
#    each gate PSUM first via identity-matmul (the start=True write of the
#    bank), then the fp8 DR terms accumulate.  Edge projection stays bf16.
#  - State hx is bf16.  Natural-layout fp8 stationaries are produced by
#    per-block XBAR DMA transposes (bf16, on the otherwise-idle DMA
#    engines) + fp8 casts -- no PE transposes, no PSUM round-trips.
#  - The recurrence is latency-bound, so emission order == readiness order
#    on every in-order engine queue: r-gates/sigmoids flow block-by-block
#    into the rh exports and mmB; the u path (sigma_u, W = u*hx, v = 1-u)
#    rides in mmB's shadow so only two DVE hops (p = v*cy, hy = W + p)
#    follow each tanh; output sigmoids land in an SBUF ring and are DMA'd
#    out two steps later (and in a final burst), keeping the out path off
#    the SP/transpose queue entirely.
#  - PSUM (8 banks): big ring 2x[128,1024]; r-gate ring 2x[128,512];
#    u-gate ring 2x[128,512]; cand + edge psums cycle through the big ring
#    between the big matmuls.
#
# kernel() takes FULL unsharded inputs, returns FULL [T, B, N, F] fp32.

import numpy as np
from contextlib import ExitStack

import concourse.bass as bass
import concourse.tile as tile
from concourse import bacc, mybir
from concourse.bass_utils import run_bass_kernel_spmd

F32 = mybir.dt.float32
BF16 = mybir.dt.bfloat16
FP8 = mybir.dt.float8e4
DR = mybir.MatmulPerfMode.DoubleRow

B, N, F = 16, 2048, 64
T = 8
NCORES = 8
BL = B // NCORES          # batches per core (2)
C = BL * F                # 128 partitions
NT = N // 128             # 16 m-tiles
NBLK = 4                  # n blocks for elementwise/small-mm work
BLK = N // NBLK           # 512
NCH = N // 256            # 8 DoubleRow n-chunks per big matmul
NPAIR = NT // 2           # 8 DoubleRow K-pairs

S_L = 64.0                # L^T fp8 pre-scale (host)
S_H = 16.0                # hx / rh fp8 cast scale (device)
S_C = 4.0                 # lxh8 scale; psum A holds S_L*S_H*lxh
G = 64.0                  # gate/cand PSUM pre-activation scale

W8_PAIRS = ["w8_r", "w8_u", "w8_c"]
WB_NAMES = ["identb", "web", "wx0r", "wx1r", "wx0u", "wx1u", "wxc0", "wxc1"]
B_NAMES = ["bgr", "bgu", "bcc", "bee"]

MUL = mybir.AluOpType.mult
ADD = mybir.AluOpType.add
SUBTRACT = mybir.AluOpType.subtract


def _emit(ctx: ExitStack, tc: tile.TileContext, d):
    nc = tc.nc
    AF = mybir.ActivationFunctionType

    consts = ctx.enter_context(tc.tile_pool(name="consts", bufs=1))
    work = ctx.enter_context(tc.tile_pool(name="work", bufs=3))
    # PSUM (8 banks): big ring 2x[128,1024] = 4 banks; r-gate ring
    # 2x[128,512] = 2 banks; u-gate ring 2x[128,512] = 2 banks.  cand and
    # edge psums cycle through the big ring between the big matmuls.
    big_ps = ctx.enter_context(tc.tile_pool(name="bigps", bufs=2, space="PSUM"))
    r_ps = ctx.enter_context(tc.tile_pool(name="rps", bufs=2, space="PSUM"))
    u_ps = ctx.enter_context(tc.tile_pool(name="ups", bufs=2, space="PSUM"))

    # ---- static loads -------------------------------------------------
    wpack8 = consts.tile([128, 6, 128], FP8, tag="wpack8")
    nc.sync.dma_start(wpack8[:], d["wpack8"][:, :].rearrange("p (i m) -> p i m", i=6))
    w8 = {name: wpack8[:, 2 * i:2 * i + 2, :] for i, name in enumerate(W8_PAIRS)}
    wpackb = consts.tile([128, len(WB_NAMES) * 128], BF16, tag="wpackb")
    nc.sync.dma_start(wpackb[:], d["wpackb"][:, :])
    wb = {name: wpackb[:, i * 128:(i + 1) * 128]
          for i, name in enumerate(WB_NAMES)}
    bpack = consts.tile([128, len(B_NAMES)], F32, tag="bpack")
    nc.sync.dma_start(bpack[:], d["bpack"][:, :])
    bias = {name: bpack[:, j:j + 1] for j, name in enumerate(B_NAMES)}

    xT = consts.tile([128, N], BF16, tag="xT")
    nc.sync.dma_start(xT[:], d["xT"][:, :])
    xnat8 = consts.tile([128, NT, 128], FP8, tag="xnat8")
    try:
        nc.sync.dma_start(xnat8[:], d["xnat8"].rearrange("(a p) c -> p a c", p=128))
    except Exception:
        for mi in range(NT):
            nc.sync.dma_start(xnat8[:, mi, :],
                              d["xnat8"][mi * 128:(mi + 1) * 128, :])

    lt8 = consts.tile([128, NT, N], FP8, tag="lt8")
    for mi in range(NT):
        nc.sync.dma_start(lt8[:, mi, :], d["lt8"][mi * 128:(mi + 1) * 128, :])

    # ---- persistent step buffers --------------------------------------
    hxbuf = [consts.tile([128, N], BF16, tag=f"hxT{i}", name=f"hxT{i}")
             for i in range(2)]
    ruT = consts.tile([128, 2, N], BF16, tag="ruT")   # r plane | u plane
    cyT = consts.tile([128, N], BF16, tag="cyT")
    rhT = consts.tile([128, N], BF16, tag="rhT")
    hynat = consts.tile([128, NT, 128], BF16, tag="hynat")
    rhnat = consts.tile([128, NT, 128], BF16, tag="rhnat")
    s8hy = consts.tile([128, NT, 128], FP8, tag="s8hy")
    s8rh = consts.tile([128, NT, 128], FP8, tag="s8rh")
    rupack = consts.tile([128, 2, N], FP8, tag="rupack")      # hxT8 | lxh8
    candpack = consts.tile([128, 2, N], FP8, tag="candpack")  # rhT8 | lrh8
    WT = consts.tile([128, N], BF16, tag="WT")
    gcr = consts.tile([128, N], BF16, tag="gcr")
    gcu = consts.tile([128, N], BF16, tag="gcu")
    gcc = consts.tile([128, N], BF16, tag="gcc")
    LxT = consts.tile([128, N], BF16, tag="LxT")
    ytbuf = consts.tile([128, T * N], BF16, tag="ytbuf")

    def nb(ap, blk):
        return ap[:, blk * BLK:(blk + 1) * BLK]

    def big_mm_half(stat8, halves, h):
        """One psum half (n-chunks [4h, 4h+4)) of the DoubleRow big matmul,
        all K-pairs in block-group order.  Emitting h=0 fully before h=1
        lets the half-0 cast + its consumers overlap the half-1 pass, while
        the input side still pipelines on the per-block stationary casts."""
        for bg in range(4):
            for p in (2 * bg, 2 * bg + 1):
                st = stat8[:, 2 * p:2 * p + 2, :]
                for j in range(4 * h, 4 * h + 4):
                    nc.tensor.matmul(
                        halves[h][:, (j % 4) * 256:(j % 4) * 256 + 256],
                        st, lt8[:, 2 * p:2 * p + 2, j * 256:(j + 1) * 256],
                        start=(p == 0 and j % 2 == 0),
                        stop=(p == 7),
                        perf_mode=DR, skip_group_check=True)

    def gate_ident(ps, gc, blk):
        nc.tensor.matmul(ps[:], wb["identb"], nb(gc, blk),
                         start=True, stop=False, skip_group_check=True)

    def gate_dr(ps, wpair, pack, blk):
        for q in range(2):
            ch = 2 * blk + q
            nc.tensor.matmul(
                ps[:, q * 256:(q + 1) * 256],
                wpair, pack[:, :, ch * 256:(ch + 1) * 256],
                start=False, stop=(q == 1),
                perf_mode=DR, skip_group_check=True)

    def cast_fp8(eng, dst, src, scale):
        if eng is nc.scalar:
            eng.mul(dst, src, float(scale))
        else:
            eng.tensor_scalar_mul(dst, src, float(scale))

    SC_LXH = S_C / (S_L * S_H)

    # =========== precompute ============================================
    pre = [big_ps.tile([128, 1024], F32, tag="big", name=f"pre{h}")
           for h in range(2)]
    for h in range(2):
        big_mm_half(xnat8, pre, h)
    cast_fp8(nc.scalar, LxT[:, 0:512], pre[0][:, 0:512], 1.0 / (S_L * S_H))
    cast_fp8(nc.scalar, LxT[:, 512:1024], pre[0][:, 512:1024], 1.0 / (S_L * S_H))
    cast_fp8(nc.scalar, LxT[:, 1024:1536], pre[1][:, 0:512], 1.0 / (S_L * S_H))
    cast_fp8(nc.scalar, LxT[:, 1536:2048], pre[1][:, 512:1024], 1.0 / (S_L * S_H))
    for blk in range(NBLK):
        for pool, tg, wa, wc_, dst, bs in (
                (r_ps, "r", "wx0r", "wx1r", gcr, "bgr"),
                (u_ps, "u", "wx0u", "wx1u", gcu, "bgu"),
                (r_ps, "r", "wxc0", "wxc1", gcc, "bcc")):
            ps = pool.tile([128, BLK], F32, tag=tg, name="cps")
            nc.tensor.matmul(ps[:], wb[wa], nb(xT, blk), start=True, stop=False,
                             skip_group_check=True)
            nc.tensor.matmul(ps[:], wb[wc_], nb(LxT, blk), start=False, stop=True,
                             skip_group_check=True)
            if pool is u_ps:
                nc.scalar.activation(nb(dst, blk), ps[:], AF.Identity,
                                     bias=bias[bs][:])
            else:
                nc.vector.scalar_tensor_tensor(
                    nb(dst, blk), ps[:], 1.0,
                    bias[bs][:].broadcast_to([128, BLK]), op0=MUL, op1=ADD)

    # =========== step 0 (hx == 0) ======================================
    hyT = hxbuf[1]
    uT0 = ruT[:, 1, :]
    for blk in range(NBLK):
        nc.scalar.activation(nb(uT0, blk), nb(gcu, blk), AF.Sigmoid,
                             scale=1.0 / G)
        nc.scalar.activation(nb(cyT, blk), nb(gcc, blk), AF.Tanh,
                             scale=1.0 / G)
        e = work.tile([128, BLK], BF16, tag="tmp", name="e0")
        nc.vector.tensor_mul(e[:], nb(uT0, blk), nb(cyT, blk))
        nc.vector.tensor_sub(nb(hyT, blk), nb(cyT, blk), e[:])
    for blk in range(NBLK):
        nc.sync.dma_start_transpose(
            hynat[:, 4 * blk:4 * (blk + 1), :], nb(hyT, blk))
    for blk in range(NBLK):
        cast_fp8(nc.vector, s8hy[:, 4 * blk:4 * (blk + 1), :],
                 hynat[:, 4 * blk:4 * (blk + 1), :], S_H)
    cast_fp8(nc.gpsimd, rupack[:, 0, :], hyT[:], S_H)
    edge0 = [big_ps.tile([128, 1024], F32, tag="big", name=f"edg0{h}")
             for h in range(2)]
    for blk in range(NBLK):
        nc.tensor.matmul(edge0[blk // 2][:, (blk % 2) * BLK:(blk % 2 + 1) * BLK],
                         wb["web"], nb(hyT, blk), start=True, stop=True,
                         skip_group_check=True)
    for blk in range(NBLK):
        nc.scalar.activation(
            ytbuf[:, blk * BLK:(blk + 1) * BLK],
            edge0[blk // 2][:, (blk % 2) * BLK:(blk % 2 + 1) * BLK],
            AF.Sigmoid, bias=bias["bee"][:])

    # =========== steps 1..T-1 ==========================================
    # Serial-chain-minimized schedule.  Per step, the critical chain is:
    #   mmA -> lxh cast -> rDR -> sigma_r -> rh mul -> xbar transpose ->
    #   s8rh cast -> mmB -> lrh cast -> candDR -> tanh -> p -> hy ->
    #   transpose -> s8hy cast -> mmA' ...
    # The u-gate path (sigma_u, W = u*hx, v = 1-u) rides in mmB's shadow,
    # so only two DVE hops (p = v*cy, hy = W + p) follow each tanh.
    rps = [None] * NBLK
    ups = [None] * NBLK

    def alloc_r(blk):
        rps[blk] = r_ps.tile([128, BLK], F32, tag="r", name="rps")
        gate_ident(rps[blk], gcr, blk)

    def alloc_u(blk):
        ups[blk] = u_ps.tile([128, BLK], F32, tag="u", name="ups")
        gate_ident(ups[blk], gcu, blk)

    rT = ruT[:, 0, :]
    uT = ruT[:, 1, :]

    for t in range(1, T):
        hxT, hyT = hxbuf[t % 2], hxbuf[(t + 1) % 2]
        # --- phase A ---------------------------------------------------
        alloc_r(0)
        alloc_r(1)
        alloc_u(0)
        alloc_u(1)
        psA = [big_ps.tile([128, 1024], F32, tag="big", name=f"psA{h}")
               for h in range(2)]
        big_mm_half(s8hy, psA, 0)
        cast_fp8(nc.vector, rupack[:, 1, 0:512], psA[0][:, 0:512], SC_LXH)
        big_mm_half(s8hy, psA, 1)
        cast_fp8(nc.vector, rupack[:, 1, 512:1024], psA[0][:, 512:1024], SC_LXH)
        cast_fp8(nc.vector, rupack[:, 1, 1024:1536], psA[1][:, 0:512], SC_LXH)
        cast_fp8(nc.vector, rupack[:, 1, 1536:2048], psA[1][:, 512:1024], SC_LXH)
        # --- phase A2: r gates -> rh -> exports; u deferred -----------
        gate_dr(rps[0], w8["w8_r"], rupack, 0)
        gate_dr(rps[1], w8["w8_r"], rupack, 1)
        alloc_r(2)
        gate_dr(rps[2], w8["w8_r"], rupack, 2)
        alloc_r(3)
        gate_dr(rps[3], w8["w8_r"], rupack, 3)
        gate_dr(ups[0], w8["w8_u"], rupack, 0)
        gate_dr(ups[1], w8["w8_u"], rupack, 1)
        for blk in range(NBLK):
            nc.scalar.activation(nb(rT, blk), rps[blk][:], AF.Sigmoid,
                                 scale=1.0 / G)
        for blk in range(NBLK):
            nc.vector.tensor_mul(nb(rhT, blk), nb(rT, blk), nb(hxT, blk))
        for blk in range(NBLK):
            nc.sync.dma_start_transpose(
                rhnat[:, 4 * blk:4 * (blk + 1), :], nb(rhT, blk))
        for blk in range(NBLK):
            cast_fp8(nc.vector, s8rh[:, 4 * blk:4 * (blk + 1), :],
                     rhnat[:, 4 * blk:4 * (blk + 1), :], S_H)
            cast_fp8(nc.gpsimd, candpack[:, 0, blk * BLK:(blk + 1) * BLK],
                     nb(rhT, blk), S_H)
        # --- phase B ---------------------------------------------------
        psB = [big_ps.tile([128, 1024], F32, tag="big", name=f"psB{h}")
               for h in range(2)]
        big_mm_half(s8rh, psB, 0)
        cast_fp8(nc.vector, candpack[:, 1, 0:1024], psB[0][:], SC_LXH)
        big_mm_half(s8rh, psB, 1)
        cast_fp8(nc.vector, candpack[:, 1, 1024:2048], psB[1][:], SC_LXH)
        for blk in (0, 1):
            nc.scalar.activation(nb(uT, blk), ups[blk][:], AF.Sigmoid,
                                 scale=1.0 / G)
        alloc_u(2)
        gate_dr(ups[2], w8["w8_u"], rupack, 2)
        alloc_u(3)
        gate_dr(ups[3], w8["w8_u"], rupack, 3)
        # blend prep (shadow) AFTER the critical lrh casts in the DVE queue
        for blk in (0, 1):
            nc.vector.tensor_mul(nb(WT, blk), nb(uT, blk), nb(hxT, blk))
            nc.vector.tensor_scalar(nb(uT, blk), nb(uT, blk), -1.0, 1.0,
                                    op0=MUL, op1=ADD)
        # --- phase B2: cand -> tanh -> (p, hy); u2/u3 woven between ----
        candps = [None, None]
        for half in range(2):
            cp = big_ps.tile([128, 1024], F32, tag="big", name=f"cand{half}")
            candps[half] = cp
            for blk in (2 * half, 2 * half + 1):
                nc.tensor.matmul(cp[:, (blk % 2) * BLK:(blk % 2 + 1) * BLK],
                                 wb["identb"], nb(gcc, blk),
                                 start=True, stop=False, skip_group_check=True)
                for q in range(2):
                    ch = 2 * blk + q
                    nc.tensor.matmul(
                        cp[:, (blk % 2) * BLK + q * 256:(blk % 2) * BLK + (q + 1) * 256],
                        w8["w8_c"], candpack[:, :, ch * 256:(ch + 1) * 256],
                        start=False, stop=(q == 1),
                        perf_mode=DR, skip_group_check=True)
        nc.scalar.activation(nb(cyT, 0), candps[0][:, 0:BLK], AF.Tanh,
                             scale=1.0 / G)
        nc.scalar.activation(nb(uT, 2), ups[2][:], AF.Sigmoid, scale=1.0 / G)
        nc.scalar.activation(nb(cyT, 1), candps[0][:, BLK:2 * BLK], AF.Tanh,
                             scale=1.0 / G)
        nc.scalar.activation(nb(uT, 3), ups[3][:], AF.Sigmoid, scale=1.0 / G)
        nc.scalar.activation(nb(cyT, 2), candps[1][:, 0:BLK], AF.Tanh,
                             scale=1.0 / G)
        nc.scalar.activation(nb(cyT, 3), candps[1][:, BLK:2 * BLK], AF.Tanh,
                             scale=1.0 / G)
        # DVE: p/hy interleaved with late W/v so nothing blocks the head
        pp = [None] * NBLK

        def blend(blk):
            p = work.tile([128, BLK], BF16, tag="tmp", name="pp")
            nc.vector.tensor_mul(p[:], nb(uT, blk), nb(cyT, blk))
            nc.vector.tensor_add(nb(hyT, blk), nb(WT, blk), p[:])

        blend(0)
        for blk in (2, 3):
            nc.vector.tensor_mul(nb(WT, blk), nb(uT, blk), nb(hxT, blk))
            nc.vector.tensor_scalar(nb(uT, blk), nb(uT, blk), -1.0, 1.0,
                                    op0=MUL, op1=ADD)
            blend(blk - 1)
        blend(3)
        if t < T - 1:
            for blk in range(NBLK):
                nc.sync.dma_start_transpose(
                    hynat[:, 4 * blk:4 * (blk + 1), :], nb(hyT, blk))
            for blk in range(NBLK):
                cast_fp8(nc.vector, s8hy[:, 4 * blk:4 * (blk + 1), :],
                         hynat[:, 4 * blk:4 * (blk + 1), :], S_H)
            cast_fp8(nc.gpsimd, rupack[:, 0, :], hyT[:], S_H)
        edgeps = [None, None]
        for half in range(2):
            ep = big_ps.tile([128, 1024], F32, tag="big", name=f"edge{half}")
            edgeps[half] = ep
            for blk in (2 * half, 2 * half + 1):
                nc.tensor.matmul(ep[:, (blk % 2) * BLK:(blk % 2 + 1) * BLK],
                                 wb["web"], nb(hyT, blk), start=True, stop=True,
                                 skip_group_check=True)
        for blk in range(NBLK):
            nc.scalar.activation(
                ytbuf[:, t * N + blk * BLK:t * N + (blk + 1) * BLK],
                edgeps[blk // 2][:, (blk % 2) * BLK:(blk % 2 + 1) * BLK],
                AF.Sigmoid, bias=bias["bee"][:])
        if t >= 2:
            nc.sync.dma_start(d["out"][t - 2, :, :], ytbuf[:, (t - 2) * N:(t - 1) * N])
        if t == T - 1:
            # out[T-2] has been ready since the previous step's sigmoids;
            # out[T-1] goes per block, each DMA chasing its own sigmoid, so
            # the final transfer tail is one [128,512] DMA, not a full row.
            nc.sync.dma_start(d["out"][T - 2, :, :],
                              ytbuf[:, (T - 2) * N:(T - 1) * N])
            for blk in range(NBLK):
                nc.sync.dma_start(
                    d["out"][T - 1, :, blk * BLK:(blk + 1) * BLK],
                    ytbuf[:, (T - 1) * N + blk * BLK:(T - 1) * N + (blk + 1) * BLK])


_BUILT = {}


def _build():
    if "nc" in _BUILT:
        return _BUILT["nc"]
    nc = bacc.Bacc("TRN2", target_bir_lowering=False, debug=False)
    d = {}
    d["lt8"] = nc.dram_tensor("lt8", [N, N], FP8, kind="ExternalInput").ap()
    d["xnat8"] = nc.dram_tensor("xnat8", [N, C], FP8, kind="ExternalInput").ap()
    d["xT"] = nc.dram_tensor("xT", [C, N], BF16, kind="ExternalInput").ap()
    d["wpack8"] = nc.dram_tensor("wpack8", [128, 6 * 128], FP8,
                                 kind="ExternalInput").ap()
    d["wpackb"] = nc.dram_tensor("wpackb", [128, len(WB_NAMES) * 128], BF16,
                                 kind="ExternalInput").ap()
    d["bpack"] = nc.dram_tensor("bpack", [128, len(B_NAMES)], F32,
                                kind="ExternalInput").ap()
    d["out"] = nc.dram_tensor("out", [T, C, N], BF16, kind="ExternalOutput").ap()

    with tile.TileContext(nc) as tc, ExitStack() as ctx:
        _emit(ctx, tc, d)
    nc.compile()
    _BUILT["nc"] = nc
    return nc


def _bd(m):
    """[64,64] -> block-diagonal [128,128] (two independent batches)."""
    z = np.zeros((128, 128), np.float32)
    z[:64, :64] = m
    z[64:, 64:] = m
    return z


def _q8(a):
    import ml_dtypes
    e4 = getattr(ml_dtypes, "float8_e4m3fn", None) or ml_dtypes.float8_e4m3
    return np.clip(np.asarray(a, np.float32), -240.0, 240.0).astype(e4)


def _bf(a):
    import ml_dtypes
    return np.asarray(a, np.float32).astype(ml_dtypes.bfloat16)


def make_in_maps(inputs_edge, L_tilde, W_gate, b_gate, W_upd, b_upd,
                 W_edge, b_edge):
    """Host-side layout transforms + quantization + per-core sharding."""
    x = np.asarray(inputs_edge, np.float32)
    L = np.asarray(L_tilde, np.float32)
    Wg0, Wg1 = np.asarray(W_gate[0], np.float32), np.asarray(W_gate[1], np.float32)
    Wu0, Wu1 = np.asarray(W_upd[0], np.float32), np.asarray(W_upd[1], np.float32)
    We = np.asarray(W_edge, np.float32)
    bg = np.asarray(b_gate, np.float32)
    bu = np.asarray(b_upd, np.float32)
    be = np.asarray(b_edge, np.float32)

    # fp8 DR weight pairs, scales folded:
    #   slot0 (vs hxT8 = S_H*hx):   (G/S_H) * wh0
    #   slot1 (vs lxh8 = S_C*lxh):  (G/S_C) * wh1
    s0, s1 = G / S_H, G / S_C
    wpack8 = np.concatenate([
        _bd(s0 * Wg0[F:, :F]), _bd(s1 * Wg1[F:, :F]),      # r
        _bd(s0 * Wg0[F:, F:]), _bd(s1 * Wg1[F:, F:]),      # u
        _bd(s0 * Wu0[F:]), _bd(s1 * Wu1[F:]),              # cand
    ], axis=1)
    wpackb = np.concatenate([
        np.eye(128, dtype=np.float32), _bd(We),
        _bd(G * Wg0[:F, :F]), _bd(G * Wg1[:F, :F]),
        _bd(G * Wg0[:F, F:]), _bd(G * Wg1[:F, F:]),
        _bd(G * Wu0[:F]), _bd(G * Wu1[:F]),
    ], axis=1)
    bpack = np.stack([G * np.tile(bg[:F], 2), G * np.tile(bg[F:], 2),
                      G * np.tile(bu, 2), np.tile(be, 2)], axis=1)
    shared = {
        "lt8": np.ascontiguousarray(_q8(S_L * L.T)),
        "wpack8": np.ascontiguousarray(_q8(wpack8)),
        "wpackb": np.ascontiguousarray(_bf(wpackb)),
        "bpack": np.ascontiguousarray(bpack.astype(np.float32)),
    }
    in_maps = []
    for core in range(NCORES):
        xs = x[core * BL:(core + 1) * BL]                    # [BL, N, F]
        m = dict(shared)
        m["xnat8"] = np.ascontiguousarray(
            _q8(S_H * xs.transpose(1, 0, 2).reshape(N, C)))
        m["xT"] = np.ascontiguousarray(
            _bf(xs.transpose(0, 2, 1).reshape(C, N)))
        in_maps.append(m)
    return in_maps


def unshard(core_outs):
    """[NCORES][T, C, N] (bf16) -> [T, B, N, F] fp32"""
    arr = np.stack([np.asarray(o, np.float32) for o in core_outs])
    return np.ascontiguousarray(
        arr.reshape(NCORES, T, BL, F, N)
           .transpose(1, 0, 2, 4, 3)
           .reshape(T, B, N, F).astype(np.float32))


def run(in_maps, **kw):
    nc = _build()
    return run_bass_kernel_spmd(nc, in_maps, list(range(NCORES)), **kw)


def kernel(inputs_edge, L_tilde, W_gate, b_gate, W_upd, b_upd, W_edge, b_edge):
    in_maps = make_in_maps(inputs_edge, L_tilde, W_gate, b_gate,
                           W_upd, b_upd, W_edge, b_edge)
    res = run(in_maps)
    return unshard([res.results[c]["out"] for c in range(NCORES)])
